# revision 15
# baseline (speedup 1.0000x reference)
"""MoE layer (24 experts, top-2 routing) on 8 Trainium2 NeuronCores.

Expert-parallel sharding: the host computes the gate routing (scores -> top-2
-> softmax combine weights), then dispatches each expert's tokens to the core
that owns the expert (3 experts per core, count-balanced by a sort-descending
assignment).  Each core runs one SPMD Bass/Tile program that, for each of its
3 expert slots, computes

    H^T[f, t] = gelu(w1^T-contract(x^T) + b1)      (MM1, K = d_model = 1024)
    Y^T[d, t] = w2^T-contract(H^T) + b2            (MM2, K = d_ff    = 4096)
    out       = Y^T * gate_weight[t]

with tokens on the matmul FREE dim, so per-expert token counts need no
128-padding (capacity = max count per slot across cores).
The host scatter-adds the per-expert outputs back into the [T, d] output
(the "combine" side of the all-to-all).

Weights and activations are bf16 (weights cast host-side, H cast by the
gelu activation's output dtype); accumulation stays fp32 in PSUM, and the
epilogue (bias + gate scaling) and the returned Y^T are fp32.  bf16 halves
the weight-stream HBM traffic vs fp32 (the previous bottleneck: ~100 MB ->
~50 MB per core per invocation), which keeps the PE continuously fed and
warm; end-to-end error vs the fp32 reference is ~3e-3, well inside 2e-2.
Matmuls run 1 row/cycle with FWL weight loads hidden behind the streams.
Weight DMAs alternate between the two HWDGE rings (SP and ACT issuing
engines).

Host-side work is routing/dispatch/combine only (index math, gather,
scatter-add); all FLOPs of the MoE layer itself (both matmuls, gelu, biases,
gate weighting) run on device.
"""

import sys

for _p in ("/opt/trn_rl_repo", "/root/.axon_site/_ro/trn_rl_repo"):
    if _p not in sys.path:
        sys.path.append(_p)

import ml_dtypes
import numpy as np

import concourse.tile as tile
from concourse import bacc, mybir
from concourse.bass_utils import run_bass_kernel_spmd

B, S, D, FF, E, TOPK = 4, 1024, 1024, 4096, 24, 2
T = B * S
P = 128
KT1 = D // P     # 8  k-subtiles for MM1
MT1 = FF // P    # 32 f-tiles (MM1 output partition tiles)
KT2 = FF // P    # 32 k-subtiles for MM2
MT2 = D // P     # 8  d-tiles (MM2 output partition tiles)
N_CORES = 8
SLOTS = E // N_CORES  # 3 experts per core

BF16 = mybir.dt.bfloat16
F32 = mybir.dt.float32
NP_BF16 = ml_dtypes.bfloat16

_program_cache: dict = {}


def _build_program(caps, loop_reps=None, bench_internal_weights=False,
                   hw_loop_reps=None):
    """One SPMD program: SLOTS expert slots with token capacities caps[j].

    loop_reps: replicate the body N times by unrolling (benchmark-only).
    hw_loop_reps: wrap the (optionally unrolled) body in an on-device For_i
    loop with N trips (benchmark-only) - thousands of reps at no compile
    cost, so wall-clock timing dominates the noisy axon relay overhead.
    bench_internal_weights: benchmark-only - weights live in internal DRAM
    scratch instead of ExternalInput so wall-clock timing excludes
    host->device shipping while keeping identical DMA traffic.
    """
    nc = bacc.Bacc("TRN2", target_bir_lowering=False, debug=False)

    wkind = "Internal" if bench_internal_weights else "ExternalInput"
    wsuff = "_int" if bench_internal_weights else ""
    # w1 is fetched two f-tiles per DMA (contiguous per partition line in
    # this layout), halving the w1 DMA/semaphore count on the PE's path.
    w1t = nc.dram_tensor("w1t" + wsuff, (SLOTS, MT1 // 2, P, 2, KT1, P), BF16,
                         kind=wkind)
    w2t = nc.dram_tensor("w2t" + wsuff, (SLOTS, MT2, P, KT2, P), BF16, kind=wkind)
    b1t = nc.dram_tensor("b1t", (SLOTS, P, MT1), F32, kind="ExternalInput")
    b2t = nc.dram_tensor("b2t", (SLOTS, P, MT2), F32, kind="ExternalInput")
    xgs = [nc.dram_tensor(f"xg{j}", (P, KT1, caps[j]), BF16, kind="ExternalInput")
           for j in range(SLOTS)]
    gws = [nc.dram_tensor(f"gw{j}", (P, caps[j]), F32, kind="ExternalInput")
           for j in range(SLOTS)]
    ygs = [nc.dram_tensor(f"yg{j}", (MT2, P, caps[j]), F32, kind="ExternalOutput")
           for j in range(SLOTS)]

    with tile.TileContext(nc) as tc:
        with tc.tile_pool(name="xg", bufs=SLOTS + 1) as xg_pool, \
             tc.tile_pool(name="gw", bufs=SLOTS + 1) as gw_pool, \
             tc.tile_pool(name="bias", bufs=SLOTS + 1) as bias_pool, \
             tc.tile_pool(name="w1", bufs=4) as w1_pool, \
             tc.tile_pool(name="w2", bufs=3) as w2_pool, \
             tc.tile_pool(name="h", bufs=MT1) as h_pool, \
             tc.tile_pool(name="epi", bufs=4) as epi_pool, \
             tc.tile_pool(name="psa", bufs=4, space="PSUM") as psa, \
             tc.tile_pool(name="psb", bufs=4, space="PSUM") as psb:

            # Next-slot w1 groups (2 f-tiles each) prefetched during the
            # previous slot's phase B (issued before the w2 DMAs so they
            # land early).
            N_PRE = 2

            def body():
                dma_rr = [0]

                def wdma(dst, src):
                    # alternate DMAs across the two HWDGE rings
                    eng = nc.scalar if (dma_rr[0] % 2) else nc.sync
                    dma_rr[0] += 1
                    eng.dma_start(dst, src)

                def preload(j):
                    # xg (the big one) on the sync ring; the small gw/bias
                    # loads on the scalar ring so they never queue behind it.
                    C = caps[j]
                    xg_sb = xg_pool.tile([P, KT1, C], BF16, tag="xg")
                    nc.sync.dma_start(xg_sb[:], xgs[j].ap()[:])
                    gw_sb = gw_pool.tile([P, C], F32, tag="gw")
                    nc.scalar.dma_start(gw_sb[:], gws[j].ap()[:])
                    b1_sb = bias_pool.tile([P, MT1], F32, tag="b1")
                    nc.scalar.dma_start(b1_sb[:], b1t.ap()[j])
                    b2_sb = bias_pool.tile([P, MT2], F32, tag="b2")
                    nc.scalar.dma_start(b2_sb[:], b2t.ap()[j])
                    return (xg_sb, gw_sb, b1_sb, b2_sb)

                # Body start, hand-scheduled: xg0 leads the sync ring; the
                # first w1 groups go on the scalar ring ahead of the small
                # gw/bias loads, so the PE can start ~3us in.
                slot_in = [None] * SLOTS
                C0 = caps[0]
                xg_sb0 = xg_pool.tile([P, KT1, C0], BF16, tag="xg")
                nc.sync.dma_start(xg_sb0[:], xgs[0].ap()[:])
                pre_w1 = []
                for mp, eng in ((0, nc.scalar), (1, nc.sync)):
                    w1_sb = w1_pool.tile([P, 2, KT1, P], BF16, tag="w1")
                    eng.dma_start(w1_sb[:], w1t.ap()[0, mp])
                    pre_w1.append(w1_sb)
                gw_sb0 = gw_pool.tile([P, C0], F32, tag="gw")
                nc.scalar.dma_start(gw_sb0[:], gws[0].ap()[:])
                b1_sb0 = bias_pool.tile([P, MT1], F32, tag="b1")
                nc.scalar.dma_start(b1_sb0[:], b1t.ap()[0])
                b2_sb0 = bias_pool.tile([P, MT2], F32, tag="b2")
                nc.scalar.dma_start(b2_sb0[:], b2t.ap()[0])
                slot_in[0] = (xg_sb0, gw_sb0, b1_sb0, b2_sb0)
                dma_rr = [0]  # next w1 group -> sync, then alternate

                for j in range(SLOTS):
                    C = caps[j]
                    xg_sb, gw_sb, b1_sb, b2_sb = slot_in[j]

                    # Phase A: H^T tiles, two 128-row f-tiles per w1 DMA.
                    h_tiles = []
                    for mg in range(MT1 // 2):
                        if mg < len(pre_w1):
                            w1_sb = pre_w1[mg]
                        else:
                            w1_sb = w1_pool.tile([P, 2, KT1, P], BF16, tag="w1")
                            wdma(w1_sb[:], w1t.ap()[j, mg])
                        # Defer the next slot's input preloads into the
                        # middle of phase A so they never stall the PE.
                        if mg == 3 and j + 1 < SLOTS:
                            slot_in[j + 1] = preload(j + 1)
                        for g in range(2):
                            m = 2 * mg + g
                            ph = psa.tile([P, C], F32, tag="psa")
                            for k in range(KT1):
                                nc.tensor.matmul(ph[:], w1_sb[:, g, k, :],
                                                 xg_sb[:, k, :],
                                                 start=(k == 0),
                                                 stop=(k == KT1 - 1))
                            h_sb = h_pool.tile([P, C], BF16, tag="h")
                            nc.scalar.activation(h_sb[:], ph[:],
                                                 mybir.ActivationFunctionType.Gelu,
                                                 bias=b1_sb[:, m:m + 1])
                            h_tiles.append(h_sb)

                    # Prefetch the next slot's first w1 groups ahead of the
                    # w2 queue so phase A of slot j+1 starts immediately.
                    pre_w1 = []
                    if j + 1 < SLOTS:
                        for mp in range(N_PRE):
                            w1_sb = w1_pool.tile([P, 2, KT1, P], BF16, tag="w1")
                            wdma(w1_sb[:], w1t.ap()[j + 1, mp])
                            pre_w1.append(w1_sb)

                    # Phase B: Y^T tiles; epilogue adds b2, scales by gate.
                    for mo in range(MT2):
                        w2_sb = w2_pool.tile([P, KT2, P], BF16, tag="w2")
                        wdma(w2_sb[:], w2t.ap()[j, mo])
                        py = psb.tile([P, C], F32, tag="psb")
                        for k in range(KT2):
                            nc.tensor.matmul(py[:], w2_sb[:, k, :], h_tiles[k][:],
                                             start=(k == 0), stop=(k == KT2 - 1))
                        yb = epi_pool.tile([P, C], F32, tag="yb")
                        nc.scalar.activation(yb[:], py[:],
                                             mybir.ActivationFunctionType.Identity,
                                             bias=b2_sb[:, mo:mo + 1])
                        yo = epi_pool.tile([P, C], F32, tag="yo")
                        nc.vector.tensor_mul(yo[:], yb[:], gw_sb[:])
                        wdma(ygs[j].ap()[mo], yo[:])

            def unrolled():
                for _ in range(loop_reps or 1):
                    body()

            if hw_loop_reps:
                with tc.For_i(0, hw_loop_reps):
                    unrolled()
            else:
                unrolled()
    nc.compile()
    return nc


def _route(x2d, gate_w, gate_b):
    """fp32 gate scores -> top-2 indices -> softmax combine weights."""
    scores = x2d @ gate_w + gate_b                               # [T, E]
    topi = np.argsort(-scores, axis=1, kind="stable")[:, :TOPK]  # [T, 2]
    topv = np.take_along_axis(scores, topi, axis=1)
    g = np.exp(topv - topv.max(axis=1, keepdims=True))
    g = g / g.sum(axis=1, keepdims=True)
    return topi, g.astype(np.float32)


def kernel(x, gate_w, gate_b, w1, b1, w2, b2):
    x = np.ascontiguousarray(np.asarray(x, dtype=np.float32))
    gate_w = np.asarray(gate_w, dtype=np.float32)
    gate_b = np.asarray(gate_b, dtype=np.float32)
    w1 = np.asarray(w1, dtype=np.float32)
    b1 = np.asarray(b1, dtype=np.float32)
    w2 = np.asarray(w2, dtype=np.float32)
    b2 = np.asarray(b2, dtype=np.float32)

    x2d = x.reshape(T, D)
    topi, gates = _route(x2d, gate_w, gate_b)

    # Token list and combine weight per expert (token order preserved).
    idx_e = [np.nonzero(topi == e)[0] for e in range(E)]
    gv_e = []
    for e in range(E):
        rows = topi == e                       # [T, 2] bool, <=1 True per row
        sel = rows.any(axis=1)
        gv_e.append(gates[sel, :][rows[sel, :]].astype(np.float32))
    counts = np.array([len(i) for i in idx_e])

    # Balance experts over (core, slot): sort by count descending; slot j
    # holds ranks [8j, 8j+8).  Slot capacity = max count in the slot
    # (this sorted grouping provably minimizes sum-of-slot-maxima).
    order = np.argsort(-counts, kind="stable")
    slot_expert = np.empty((N_CORES, SLOTS), dtype=int)
    caps = []
    for j in range(SLOTS):
        ranks = order[j * N_CORES:(j + 1) * N_CORES]
        slot_expert[:, j] = ranks
        caps.append(max(int(counts[ranks].max()), 2))
    caps = tuple(caps)

    if caps not in _program_cache:
        _program_cache[caps] = _build_program(caps)
    nc = _program_cache[caps]

    xT = np.ascontiguousarray(x2d.T).astype(NP_BF16)       # [D, T] bf16
    in_maps = []
    for c in range(N_CORES):
        m = {}
        w1c = np.empty((SLOTS, MT1 // 2, P, 2, KT1, P), NP_BF16)
        w2c = np.empty((SLOTS, MT2, P, KT2, P), NP_BF16)
        b1c = np.empty((SLOTS, P, MT1), np.float32)
        b2c = np.empty((SLOTS, P, MT2), np.float32)
        for j in range(SLOTS):
            e = int(slot_expert[c, j])
            C = caps[j]
            n = int(counts[e])
            xg = np.zeros((P, KT1, C), NP_BF16)
            xg[:, :, :n] = xT[:, idx_e[e]].reshape(KT1, P, n).transpose(1, 0, 2)
            m[f"xg{j}"] = xg
            gw = np.zeros((C,), np.float32)
            gw[:n] = gv_e[e]
            m[f"gw{j}"] = np.broadcast_to(gw, (P, C)).copy()
            # weight tiles in the exact SBUF layouts for single clean DMAs
            w1c[j] = (w1[e].reshape(KT1, P, MT1, P).transpose(2, 1, 0, 3)
                      .reshape(MT1 // 2, 2, P, KT1, P).transpose(0, 2, 1, 3, 4)
                      .astype(NP_BF16))
            w2c[j] = w2[e].reshape(KT2, P, MT2, P).transpose(2, 1, 0, 3).astype(NP_BF16)
            b1c[j] = b1[e].reshape(MT1, P).T
            b2c[j] = b2[e].reshape(MT2, P).T
        m["w1t"] = w1c
        m["w2t"] = w2c
        m["b1t"] = b1c
        m["b2t"] = b2c
        in_maps.append(m)

    res = run_bass_kernel_spmd(nc, in_maps, core_ids=list(range(N_CORES)))

    # Combine: scatter-add each expert's weighted outputs back to tokens.
    out = np.zeros((T, D), np.float32)
    for c in range(N_CORES):
        for j in range(SLOTS):
            e = int(slot_expert[c, j])
            n = int(counts[e])
            yg = res.results[c][f"yg{j}"].reshape(D, caps[j])
            out[idx_e[e], :] += yg[:, :n].T
    return out.reshape(B, S, D)
